# revision 22
# baseline (speedup 1.0000x reference)
# Discrete-Hawkes kernel for Trainium2 (8 NeuronCores, SPMD, no collectives).
#
# lam(t,s) = relu( mu[s] + beta * H[t,s] ),
#   H[t] = a*(H[t-1] + c[t-1]),  c = obs @ alpha,  a = exp(-beta)
#
# Layout: everything transposed ([space -> partitions, time -> free]) so that
#  * cT = alpha^T @ obsT is an fp8 x fp8 GEMM run in DoubleRow perf mode
#    (contraction 256 per matmul, ~2x the bf16 column rate),
#  * the time recurrence is a DVE tensor_tensor_scan per 128-space tile
#    (state = a*state + c[t-1], streamed along the free axis),
#  * relu(beta*H + mu) fuses into ONE activation op (mu and beta*a are
#    per-partition scalars in this layout).
#
# Sharding: time is split across the 8 cores (1024 steps each). Instead of a
# recomputed history halo, each core's scan state is seeded with
# s = H[t_start]/a, precomputed on the host from the trailing 512 steps of obs
# (a few MFLOP, exact to f32) and passed in via the consts tensor. Column 0 of
# each core's lambda grid (which depends only on the seed) is also patched on
# the host, so the device only computes columns 1..1023. The final [B]-point
# gather of the fp16 lambda grid happens on host.

import numpy as np
import ml_dtypes

T, S, B = 8192, 1024, 8192
NCORES = 8
TLOC = T // NCORES          # 1024 time columns owned per core
P = 128
KT = S // P                 # 8 contraction tiles of 128
MT = S // P                 # 8 space tiles of 128
CH = 2                      # 512-column matmul chunks (one PSUM bank each)
W = TLOC // CH              # 512
SEED_WIN = 512              # host-side history window for the seed state

_NC_CACHE = {}
LAST_RESULT = None          # BassKernelResults of the most recent run


def _build():
    if "nc" in _NC_CACHE:
        return _NC_CACHE["nc"]

    import concourse.mybir as mybir
    import concourse.tile as tile
    from concourse import bacc

    dt = mybir.dt
    nc = bacc.Bacc("TRN2", target_bir_lowering=False, debug=False,
                   num_devices=NCORES)

    # obst pre-arranged on host as [p, ch, kk, w] = obsT[kk*128+p, ch*512+w]
    # (4 KB contiguous per (partition, chunk) -> large DMA descriptors, and
    # each 512-column chunk can be DMA'd -- and its PSUM bank scanned -- alone)
    obst_d = nc.dram_tensor("obst", [P, CH, KT, W], dt.float8e4,
                            kind="ExternalInput")
    # alpha pre-arranged on host as [p, m, kk, j] = alpha[kk*128+p, m*128+j]
    # (per-m slices are 1 KB contiguous per partition -> one small DMA per m)
    alpha_d = nc.dram_tensor("alpha", [P, MT, KT, P], dt.float8e4,
                             kind="ExternalInput")
    # consts: col0 = a, col1 = beta*a, cols 2..9 = mu tiles, 10..17 = scan seed
    consts_d = nc.dram_tensor("consts", [P, 2 + 2 * MT], dt.float32,
                              kind="ExternalInput")
    lamt_d = nc.dram_tensor("lamt", [S, TLOC], dt.float16,
                            kind="ExternalOutput")

    DR = mybir.MatmulPerfMode.DoubleRow

    with tile.TileContext(nc) as tc:
        with (
            tc.tile_pool(name="inp", bufs=1) as inp,
            tc.tile_pool(name="psum", bufs=3, space="PSUM") as psum,
            tc.tile_pool(name="work", bufs=2) as work,
            tc.tile_pool(name="outp", bufs=2) as outp,
        ):
            consts_sb = inp.tile([P, 2 + 2 * MT], dt.float32, tag="consts")
            alpha_sb = inp.tile([P, KT, MT * P], dt.float8e4, tag="alpha")
            obst_sb = inp.tile([P, CH, KT, W], dt.float8e4, tag="obst")

            # All input DMAs FIFO on the sync queue pool (one pool keeps the
            # descriptor generator pipelined at full HBM rate), ordered so the
            # kk-pairs gating the first matmuls land first. consts ride the
            # otherwise-idle vector pool.
            nc.scalar.dma_start(consts_sb[:], consts_d[:, :])
            nc.sync.dma_start(alpha_sb[:, 0:2, :], alpha_d[:, 0:2, :])
            nc.sync.dma_start(obst_sb[:, 0:4, :], obst_d[:, 0:4, :])
            nc.sync.dma_start(alpha_sb[:, 2:6, :], alpha_d[:, 2:6, :])
            nc.sync.dma_start(obst_sb[:, 4:, :], obst_d[:, 4:, :])
            nc.sync.dma_start(alpha_sb[:, 6:, :], alpha_d[:, 6:, :])

            # PE warm-up: a string of tiny dependency-free matmuls bridges the
            # idle window between the engine preamble and the first real
            # matmul, so HAM throttling has ramped utilization back up before
            # the GEMM stream starts.


            wsb = inp.tile([P, 8], dt.bfloat16, tag="wsb")
            wsb2 = inp.tile([P, 8], dt.bfloat16, tag="wsb2")
            nc.gpsimd.memset(wsb[:], 1.0)
            nc.gpsimd.memset(wsb2[:], 1.0)
            warm = warmp.tile([P, 8], dt.float32, tag="warm", name="warm")
            for _ in range(200):
                nc.tensor.matmul(warm[0:8, 0:8], wsb[:, 0:8], wsb2[:, 0:8],
                                 start=True, stop=True)

            a_ap = consts_sb[:, 0:1]        # exp(-beta), per-partition scalar
            ab_ap = consts_sb[:, 1:2]       # beta * exp(-beta)

            def emit_mms(m, ps, ch):
                for k in range(KT // 2):
                    lhsT = alpha_sb[:, m, 2 * k:2 * k + 2, :]
                    nc.tensor.matmul(ps[:, ch * W:(ch + 1) * W], lhsT,
                                     obst_sb[:, ch, 2 * k:2 * k + 2, :],
                                     start=(k == 0), stop=(k == KT // 2 - 1),
                                     perf_mode=DR)

            def emit_scan(m, ps, ht, lam, lo, hi, first):
                # s[t] = a*s[t-1] + c[t-1]  (then H = a*s), reading c straight
                # out of PSUM; lam = relu( (beta*a)*s + mu ).  Column 0 is
                # patched on host.
                mu_ap = consts_sb[:, 2 + m:3 + m]
                seed_ap = consts_sb[:, 2 + MT + m:3 + MT + m]
                nc.vector.tensor_tensor_scan(
                    ht[:, lo:hi],
                    a_ap.to_broadcast((P, hi - lo)),
                    ps[:, lo - 1:hi - 1],
                    seed_ap if first else ht[:, lo - 1:lo],
                    mybir.AluOpType.mult, mybir.AluOpType.add)
                nc.scalar.activation(lam[:, lo:hi], ht[:, lo:hi],
                                     mybir.ActivationFunctionType.Relu,
                                     bias=mu_ap, scale=ab_ap)
                nc.scalar.dma_start(lamt_d[m * P:(m + 1) * P, lo:hi],
                                    lam[:, lo:hi])

            for m in range(MT):
                # One 2-bank PSUM tile per m; each 512-col chunk is one bank.
                ps = psum.tile([P, TLOC], dt.float32, tag="ps", name=f"ps_{m}")
                ht = work.tile([P, TLOC], dt.float32, tag="ht")
                lam = outp.tile([P, TLOC], dt.float16, tag="lam")
                if m < 2:
                    # Interleave the scan piece of each PSUM bank right after
                    # that bank's matmul group so it gates on only those four
                    # matmuls -- this is what lets the serial scan chain start
                    # while the input DMAs are still streaming.
                    for ch in range(CH):
                        emit_mms(m, ps, ch)
                        lo, hi = (1, W + 1) if ch == 0 else (W + 1, TLOC)
                        emit_scan(m, ps, ht, lam, lo, hi, ch == 0)
                else:
                    for ch in range(CH):
                        emit_mms(m, ps, ch)
                    if m < MT - 1:
                        emit_scan(m, ps, ht, lam, 1, TLOC, True)
                    else:
                        # short last piece so the final act+store tail is small
                        emit_scan(m, ps, ht, lam, 1, 769, True)
                        emit_scan(m, ps, ht, lam, 769, TLOC, False)

    nc.compile()
    _NC_CACHE["nc"] = nc
    return nc


def _prep_inputs(obs, alpha, beta, mu):
    fp8 = ml_dtypes.float8_e4m3fn
    obs = np.asarray(obs)
    alpha32 = np.asarray(alpha, dtype=np.float32)
    # [p, m, kk, j] = alpha[kk*128+p, m*128+j]
    alpha_b = np.ascontiguousarray(
        alpha32.astype(fp8).reshape(KT, P, MT, P).transpose(1, 2, 0, 3))
    beta32 = np.float32(np.asarray(beta).reshape(-1)[0])
    a32 = np.exp(-beta32, dtype=np.float32)
    mu32 = np.asarray(mu, dtype=np.float32)

    # Scan seed per core: s = H[t_start]/a = sum_{d>=1} a^(d-1) c[t_start-d],
    # computed on host from the trailing SEED_WIN observation rows (exact to
    # f32: a^512 underflows long before that).
    a64 = np.exp(-np.float64(beta32))
    wvec = a64 ** np.arange(SEED_WIN, dtype=np.float64)   # a^(d-1), d=1..WIN
    alpha_q64 = alpha32.astype(fp8).astype(np.float64)
    seeds = np.zeros((NCORES, S), dtype=np.float32)
    for k in range(1, NCORES):
        start = k * TLOC
        win = obs[start - SEED_WIN:start][::-1].astype(np.float64)  # [d-1, sp]
        g = wvec @ win                                    # [S] weighted obs
        seeds[k] = (g @ alpha_q64).astype(np.float32)

    obs8 = obs.T.astype(fp8).reshape(KT, P, T)            # [kk, p, t]

    consts = np.zeros((P, 2 + 2 * MT), dtype=np.float32)
    consts[:, 0] = a32
    consts[:, 1] = np.float32(beta32 * a32)
    consts[:, 2:2 + MT] = mu32.reshape(MT, P).T

    in_maps = []
    for k in range(NCORES):
        obst_k = np.ascontiguousarray(
            obs8[:, :, k * TLOC:(k + 1) * TLOC]
            .reshape(KT, P, CH, W).transpose(1, 2, 0, 3))
        consts_k = consts.copy()
        consts_k[:, 2 + MT:] = seeds[k].reshape(MT, P).T
        in_maps.append({"obst": obst_k, "alpha": alpha_b,
                        "consts": consts_k})

    # lam at column 0 of each core (t = k*TLOC) depends only on the seed:
    # lam = relu(mu + (beta*a) * seed); computed here and patched into the
    # gathered output on host.
    lam0 = np.maximum(
        mu32[None, :] + np.float32(beta32 * a32) * seeds, 0.0)  # [8, S]
    return in_maps, lam0


def kernel(t, s, obs, alpha, beta, mu):
    global LAST_RESULT
    from concourse import bass_utils

    nc = _build()
    in_maps, lam0 = _prep_inputs(obs, alpha, beta, mu)
    res = bass_utils.run_bass_kernel_spmd(nc, in_maps,
                                          core_ids=list(range(NCORES)))
    LAST_RESULT = res

    lam_all = np.stack([r["lamt"] for r in res.results])   # [8, S, TLOC] fp16
    t_i = np.asarray(t, dtype=np.int64)
    s_i = np.asarray(s, dtype=np.int64)
    core = t_i // TLOC
    col = t_i % TLOC
    out = lam_all[core, s_i, col].astype(np.float32)
    at0 = col == 0
    out[at0] = lam0[core[at0], s_i[at0]]
    return np.ascontiguousarray(out)


# revision 23
# speedup vs baseline: 1.1016x; 1.1016x over previous
# Discrete-Hawkes kernel for Trainium2 (8 NeuronCores, SPMD, no collectives).
#
# lam(t,s) = relu( mu[s] + beta * H[t,s] ),
#   H[t] = a*(H[t-1] + c[t-1]),  c = obs @ alpha,  a = exp(-beta)
#
# Layout: everything transposed ([space -> partitions, time -> free]) so that
#  * cT = alpha^T @ obsT is an fp8 x fp8 GEMM run in DoubleRow perf mode
#    (contraction 256 per matmul, ~2x the bf16 column rate),
#  * the time recurrence is a DVE tensor_tensor_scan per 128-space tile
#    (state = a*state + c[t-1], streamed along the free axis),
#  * relu(beta*H + mu) fuses into ONE activation op (mu and beta*a are
#    per-partition scalars in this layout).
#
# Sharding: time is split across the 8 cores (1024 steps each). Instead of a
# recomputed history halo, each core's scan state is seeded with
# s = H[t_start]/a, precomputed on the host from the trailing 512 steps of obs
# (a few MFLOP, exact to f32) and passed in via the consts tensor. Column 0 of
# each core's lambda grid (which depends only on the seed) is also patched on
# the host, so the device only computes columns 1..1023. The final [B]-point
# gather of the fp16 lambda grid happens on host.

import numpy as np
import ml_dtypes

T, S, B = 8192, 1024, 8192
NCORES = 8
TLOC = T // NCORES          # 1024 time columns owned per core
P = 128
KT = S // P                 # 8 contraction tiles of 128
MT = S // P                 # 8 space tiles of 128
CH = 2                      # 512-column matmul chunks (one PSUM bank each)
W = TLOC // CH              # 512
SEED_WIN = 512              # host-side history window for the seed state

_NC_CACHE = {}
LAST_RESULT = None          # BassKernelResults of the most recent run


def _build():
    if "nc" in _NC_CACHE:
        return _NC_CACHE["nc"]

    import concourse.mybir as mybir
    import concourse.tile as tile
    from concourse import bacc

    dt = mybir.dt
    nc = bacc.Bacc("TRN2", target_bir_lowering=False, debug=False,
                   num_devices=NCORES)

    # obst pre-arranged on host as [p, ch, kk, w] = obsT[kk*128+p, ch*512+w]
    # (4 KB contiguous per (partition, chunk) -> large DMA descriptors, and
    # each 512-column chunk can be DMA'd -- and its PSUM bank scanned -- alone)
    obst_d = nc.dram_tensor("obst", [P, CH, KT, W], dt.float8e4,
                            kind="ExternalInput")
    # alpha pre-arranged on host as [p, m, kk, j] = alpha[kk*128+p, m*128+j]
    # (per-m slices are 1 KB contiguous per partition -> one small DMA per m)
    alpha_d = nc.dram_tensor("alpha", [P, MT, KT, P], dt.float8e4,
                             kind="ExternalInput")
    # consts: col0 = a, col1 = beta*a, cols 2..9 = mu tiles, 10..17 = scan seed
    consts_d = nc.dram_tensor("consts", [P, 2 + 2 * MT], dt.float32,
                              kind="ExternalInput")
    lamt_d = nc.dram_tensor("lamt", [S, TLOC], dt.float16,
                            kind="ExternalOutput")

    DR = mybir.MatmulPerfMode.DoubleRow

    with tile.TileContext(nc) as tc:
        with (
            tc.tile_pool(name="inp", bufs=1) as inp,
            tc.tile_pool(name="psum", bufs=2, space="PSUM") as psum,
            tc.tile_pool(name="psumS", bufs=2, space="PSUM") as psums,
            tc.tile_pool(name="work", bufs=2) as work,
            tc.tile_pool(name="outp", bufs=2) as outp,
        ):
            consts_sb = inp.tile([P, 2 + 2 * MT], dt.float32, tag="consts")
            alpha_sb = inp.tile([P, KT, MT * P], dt.float8e4, tag="alpha")
            obst_sb = inp.tile([P, CH, KT, W], dt.float8e4, tag="obst")

            # All input DMAs FIFO on the sync queue pool (one pool keeps the
            # descriptor generator pipelined at full HBM rate), ordered so the
            # kk-pairs gating the first matmuls land first. consts ride the
            # otherwise-idle vector pool.
            nc.scalar.dma_start(consts_sb[:], consts_d[:, :])
            nc.sync.dma_start(alpha_sb[:, 0:2, :], alpha_d[:, 0:2, :])
            nc.sync.dma_start(obst_sb[:, 0:4, :], obst_d[:, 0:4, :])
            nc.sync.dma_start(alpha_sb[:, 2:6, :], alpha_d[:, 2:6, :])
            nc.sync.dma_start(obst_sb[:, 4:, :], obst_d[:, 4:, :])
            nc.sync.dma_start(alpha_sb[:, 6:, :], alpha_d[:, 6:, :])

            # PE warm-up: a string of tiny dependency-free matmuls bridges the
            # idle window between the engine preamble and the first real
            # matmul, so HAM throttling has ramped utilization back up before
            # the GEMM stream starts.


            wsb = inp.tile([P, 8], dt.bfloat16, tag="wsb")
            wsb2 = inp.tile([P, 8], dt.bfloat16, tag="wsb2")
            nc.gpsimd.memset(wsb[:], 1.0)
            nc.gpsimd.memset(wsb2[:], 1.0)
            warm = warmp.tile([P, 8], dt.float32, tag="warm", name="warm")
            for _ in range(200):
                nc.tensor.matmul(warm[0:8, 0:8], wsb[:, 0:8], wsb2[:, 0:8],
                                 start=True, stop=True)

            a_ap = consts_sb[:, 0:1]        # exp(-beta), per-partition scalar
            ab_ap = consts_sb[:, 1:2]       # beta * exp(-beta)

            def emit_mms(m, ps, ch):
                for k in range(KT // 2):
                    lhsT = alpha_sb[:, m, 2 * k:2 * k + 2, :]
                    nc.tensor.matmul(ps[:, :], lhsT,
                                     obst_sb[:, ch, 2 * k:2 * k + 2, :],
                                     start=(k == 0), stop=(k == KT // 2 - 1),
                                     perf_mode=DR)

            def emit_scan(m, ps_ap, ht, lam, lo, hi, first):
                # s[t] = a*s[t-1] + c[t-1]  (then H = a*s), reading c straight
                # out of PSUM; lam = relu( (beta*a)*s + mu ).  Column 0 is
                # patched on host.
                mu_ap = consts_sb[:, 2 + m:3 + m]
                seed_ap = consts_sb[:, 2 + MT + m:3 + MT + m]
                nc.vector.tensor_tensor_scan(
                    ht[:, lo:hi],
                    a_ap.to_broadcast((P, hi - lo)),
                    ps_ap,
                    seed_ap if first else ht[:, lo - 1:lo],
                    mybir.AluOpType.mult, mybir.AluOpType.add)
                nc.scalar.activation(lam[:, lo:hi], ht[:, lo:hi],
                                     mybir.ActivationFunctionType.Relu,
                                     bias=mu_ap, scale=ab_ap)
                nc.scalar.dma_start(lamt_d[m * P:(m + 1) * P, lo:hi],
                                    lam[:, lo:hi])

            def emit_mms2(m, ps, ch):
                for k in range(KT // 2):
                    lhsT = alpha_sb[:, m, 2 * k:2 * k + 2, :]
                    nc.tensor.matmul(ps[:, ch * W:(ch + 1) * W], lhsT,
                                     obst_sb[:, ch, 2 * k:2 * k + 2, :],
                                     start=(k == 0), stop=(k == KT // 2 - 1),
                                     perf_mode=DR)

            for m in range(MT):
                ht = work.tile([P, TLOC], dt.float32, tag="ht")
                lam = outp.tile([P, TLOC], dt.float16, tag="lam")
                if m < 2:
                    # Early m-tiles: one PSUM tile (bank) per 512-col chunk so
                    # each chunk's scan piece gates on only its own four
                    # matmuls -- the serial scan chain starts while the input
                    # DMAs are still streaming. Chunk boundary: scan piece A
                    # covers cols 1..512 (reads psA cols 0..511), piece B
                    # covers 513..1023 (reads psB cols 0..510).
                    psa = psums.tile([P, W], dt.float32, tag="psA",
                                     name=f"psA_{m}")
                    psb = psums.tile([P, W], dt.float32, tag="psB",
                                     name=f"psB_{m}")
                    emit_mms(m, psa, 0)
                    emit_mms(m, psb, 1)
                    emit_scan(m, psa[:, 0:W], ht, lam, 1, W + 1, True)
                    emit_scan(m, psb[:, 0:W - 1], ht, lam, W + 1, TLOC, False)
                else:
                    ps = psum.tile([P, TLOC], dt.float32, tag="ps",
                                   name=f"ps_{m}")
                    for ch in range(CH):
                        emit_mms2(m, ps, ch)
                    if m < MT - 1:
                        emit_scan(m, ps[:, 0:TLOC - 1], ht, lam, 1, TLOC, True)
                    else:
                        # short last piece so the final act+store tail is small
                        emit_scan(m, ps[:, 0:768], ht, lam, 1, 769, True)
                        emit_scan(m, ps[:, 768:TLOC - 1], ht, lam, 769, TLOC,
                                  False)
    nc.compile()
    _NC_CACHE["nc"] = nc
    return nc


def _prep_inputs(obs, alpha, beta, mu):
    fp8 = ml_dtypes.float8_e4m3fn
    obs = np.asarray(obs)
    alpha32 = np.asarray(alpha, dtype=np.float32)
    # [p, m, kk, j] = alpha[kk*128+p, m*128+j]
    alpha_b = np.ascontiguousarray(
        alpha32.astype(fp8).reshape(KT, P, MT, P).transpose(1, 2, 0, 3))
    beta32 = np.float32(np.asarray(beta).reshape(-1)[0])
    a32 = np.exp(-beta32, dtype=np.float32)
    mu32 = np.asarray(mu, dtype=np.float32)

    # Scan seed per core: s = H[t_start]/a = sum_{d>=1} a^(d-1) c[t_start-d],
    # computed on host from the trailing SEED_WIN observation rows (exact to
    # f32: a^512 underflows long before that).
    a64 = np.exp(-np.float64(beta32))
    wvec = a64 ** np.arange(SEED_WIN, dtype=np.float64)   # a^(d-1), d=1..WIN
    alpha_q64 = alpha32.astype(fp8).astype(np.float64)
    seeds = np.zeros((NCORES, S), dtype=np.float32)
    for k in range(1, NCORES):
        start = k * TLOC
        win = obs[start - SEED_WIN:start][::-1].astype(np.float64)  # [d-1, sp]
        g = wvec @ win                                    # [S] weighted obs
        seeds[k] = (g @ alpha_q64).astype(np.float32)

    obs8 = obs.T.astype(fp8).reshape(KT, P, T)            # [kk, p, t]

    consts = np.zeros((P, 2 + 2 * MT), dtype=np.float32)
    consts[:, 0] = a32
    consts[:, 1] = np.float32(beta32 * a32)
    consts[:, 2:2 + MT] = mu32.reshape(MT, P).T

    in_maps = []
    for k in range(NCORES):
        obst_k = np.ascontiguousarray(
            obs8[:, :, k * TLOC:(k + 1) * TLOC]
            .reshape(KT, P, CH, W).transpose(1, 2, 0, 3))
        consts_k = consts.copy()
        consts_k[:, 2 + MT:] = seeds[k].reshape(MT, P).T
        in_maps.append({"obst": obst_k, "alpha": alpha_b,
                        "consts": consts_k})

    # lam at column 0 of each core (t = k*TLOC) depends only on the seed:
    # lam = relu(mu + (beta*a) * seed); computed here and patched into the
    # gathered output on host.
    lam0 = np.maximum(
        mu32[None, :] + np.float32(beta32 * a32) * seeds, 0.0)  # [8, S]
    return in_maps, lam0


def kernel(t, s, obs, alpha, beta, mu):
    global LAST_RESULT
    from concourse import bass_utils

    nc = _build()
    in_maps, lam0 = _prep_inputs(obs, alpha, beta, mu)
    res = bass_utils.run_bass_kernel_spmd(nc, in_maps,
                                          core_ids=list(range(NCORES)))
    LAST_RESULT = res

    lam_all = np.stack([r["lamt"] for r in res.results])   # [8, S, TLOC] fp16
    t_i = np.asarray(t, dtype=np.int64)
    s_i = np.asarray(s, dtype=np.int64)
    core = t_i // TLOC
    col = t_i % TLOC
    out = lam_all[core, s_i, col].astype(np.float32)
    at0 = col == 0
    out[at0] = lam0[core[at0], s_i[at0]]
    return np.ascontiguousarray(out)
